# revision 1
# baseline (speedup 1.0000x reference)
"""CostVolume2D Trainium2 kernel.

out[b, d, h, w] = mean_c l[b,c,h,w] * r_pad[b,c,h, w + maxd - (d - maxd)]
               = mean_c l[b,c,h,w] * r[b,c,h, w - (d - maxd)]   (zero padded)

Strategy (8 NeuronCores, shard H — no halo since shifts only touch W):
  * Per (b, h): the 97 disparity planes are the diagonals of banded gram
    matrices G[w, w'] = sum_c l[c, w] r[c, w'] with |w - w'| <= 48.
  * Tensor engine computes G in [128 x 256] blocks (f32r, N=256 -> full rate):
      lhsT = l[:, w0:w0+128]  (K=64 channels on partitions)
      rhs  = r_padded[:, w0-48 : w0+208]
  * DVE evicts the needed 224 columns of each PSUM block to SBUF.
  * The skew (diagonal extraction) rides the store DMA: row i of a gram
    block holds the 97 output values for w = w0 + i *contiguously*
    (cols [i, i+97)), so a DMA with a joint partition+byte stride
    (flat stride = row_pitch + 1) writes output laid out as
    O[b, h, w, k] with k = maxd*2 - d_idx. Host unshards with a
    flip + transpose (pure layout glue).
  * Host pre-divides l by C (exact, power of two) so no on-device scaling,
    and pre-pads r along W so no on-device memset / edge handling.
"""

import sys

try:
    import concourse  # noqa: F401
except ImportError:
    sys.path.insert(0, "/opt/trn_rl_repo")

import numpy as np

from concourse import bass, mybir
from concourse import tile
from concourse.ap import AP
from concourse.bass_utils import run_bass_kernel_spmd

F32 = mybir.dt.float32
F32R = mybir.dt.float32r
F16 = mybir.dt.float16

# Problem dims (hardcoded per spec)
B, C, H, W = 4, 64, 256, 512
MAXD = 48
D = 2 * MAXD + 1          # 97 disparity planes
NCORES = 8
HS = H // NCORES          # 32 h-rows per core

# Derived tiling constants
WB = 128                  # w-block (gram rows per block)
NQ = W // WB              # 4 w-blocks
GW = WB + 2 * MAXD        # 224 gram columns per block
NMM = GW                  # matmul moving dim (bf16: no f32r N>=256 rule)
RPAD_L = MAXD             # left zero pad of r
RPAD_R = NMM - WB - MAXD  # 48: right pad so q=3's window is in bounds
WP = W + RPAD_L + RPAD_R  # 640 padded r width
HGRP = 8                  # h-rows loaded per input DMA (must divide HS, %2==0)
OROW = 1024               # out slots per w-row (written with pitch OROW-1)

# module-level result stash (test.py reads these)
LAST_RESULTS = None
_NC_CACHE = {}


WLR = W + WP              # 1152: combined (l | r_pad) row width


def _build_nc(b_n=B, hs=HS, hgrp=HGRP):
    """Build the per-core Bass program. All cores run the same program."""
    assert hs % hgrp == 0 and hgrp % 2 == 0
    nc = bass.Bass()
    # l and r_pad concatenated on the W axis -> ONE load DMA per h-half,
    # so every matmul depends on a single DMA semaphore lane (the f32r
    # self-loading Matmult instruction only has room for one sync wait).
    lr_in = nc.dram_tensor("lr", [b_n, C, hs, WLR], F16, kind="ExternalInput")
    o_out = nc.dram_tensor("o", [b_n, hs, WB, OROW], F16, kind="ExternalOutput")

    lr_c, lr_h = hs * WLR, WLR
    lr_b = C * hs * WLR

    n4 = hgrp // 2            # h-pairs per group
    lrw = n4 * WLR            # free width of lr tile
    gp_pitch = NQ * GW        # 896: g tile row pitch

    with tile.TileContext(nc) as tc:
        with (
            tc.tile_pool(name="lrpool", bufs=3) as lrp,
            tc.tile_pool(name="gpool", bufs=8) as gp,
            tc.tile_pool(name="ppool", bufs=8, space="PSUM") as pp,
        ):
            for b in range(b_n):
                for hg in range(hs // hgrp):
                    h0 = hg * hgrp
                    lr_t = lrp.tile([128, lrw], F16, name="lr_t")
                    # partitions = (hh in 2) x (c in 64); free = (h4, w_lr)
                    # DMA APs are limited to 3 dims -> one DMA per hh half.
                    for hh in range(2):
                        lr_src = AP(
                            lr_in, b * lr_b + (h0 + hh) * lr_h,
                            [(lr_c, C), (2 * lr_h, n4), (1, WLR)],
                        )
                        nc.sync.dma_start(
                            out=lr_t[64 * hh:64 * hh + 64, :], in_=lr_src
                        )
                    for h4 in range(n4):
                        g0 = gp.tile([128, gp_pitch], F16, name="g0", tag="g")
                        g1 = gp.tile([128, gp_pitch], F16, name="g1", tag="g")
                        gs = (g0, g1)
                        for qp in range(NQ // 2):
                            for hh in range(2):
                                p_t = pp.tile([128, 2 * NMM], F32, name="p_t")
                                for qq in range(2):
                                    q = 2 * qp + qq
                                    lhsT = lr_t[
                                        64 * hh:64 * hh + 64,
                                        h4 * WLR + WB * q:
                                        h4 * WLR + WB * q + WB,
                                    ]
                                    rhs = lr_t[
                                        64 * hh:64 * hh + 64,
                                        h4 * WLR + W + WB * q:
                                        h4 * WLR + W + WB * q + NMM,
                                    ]
                                    nc.tensor.matmul(
                                        p_t[:, NMM * qq:NMM * qq + NMM],
                                        lhsT, rhs, start=True, stop=True,
                                    )
                                nc.vector.tensor_copy(
                                    gs[hh][:, 2 * GW * qp: 2 * GW * qp + 2 * GW],
                                    p_t[:],
                                )
                        for hh in range(2):
                            h = h0 + 2 * h4 + hh
                            g = gs[hh]
                            # Full-row skew store: one descriptor per gram row
                            # (3584B). DRAM row pitch is OROW-1 elements, so
                            # row i's data lands shifted by -i: the diagonal
                            # relayout happens in the DRAM addressing, and the
                            # writes tile the region exactly (no overlap).
                            # Valid data sits at fixed slots 127+224q+k'.
                            d_ap = AP(
                                o_out,
                                (b * hs + h) * WB * OROW + (WB - 1),
                                [(OROW - 1, WB), (1, gp_pitch)],
                            )
                            eng = nc.sync if (hh % 2) else nc.scalar
                            eng.dma_start(out=d_ap, in_=g[:, :])
    _split_multi_waits(nc)
    return nc


def _split_multi_waits(nc):
    """The 64-byte TPB instruction encoding holds a single semaphore wait;
    walrus codegen rejects instructions whose sync_info carries more. Hoist
    all but one wait onto standalone InstEventSemaphore instructions placed
    immediately before, on the same engine (FIFO order preserves semantics).
    """
    for bb in nc.main_func.blocks:
        new_list = []
        changed = False
        for ins in bb.instructions:
            si = ins.sync_info
            if si is not None and len(si.on_wait) > 1:
                for w in list(si.on_wait)[:-1]:
                    ev = mybir.InstEventSemaphore(
                        name=nc.get_next_instruction_name(),
                        engine=ins.engine,
                        ins=[],
                        outs=[],
                        sync_info=mybir.SyncInfo(on_wait=[w], on_update=[]),
                    )
                    new_list.append(ev)
                ins.sync_info = mybir.SyncInfo(
                    on_wait=[list(si.on_wait)[-1]], on_update=list(si.on_update)
                )
                changed = True
            new_list.append(ins)
        if changed:
            bb.instructions = new_list


def _get_nc(key=(B, HS, HGRP)):
    if key not in _NC_CACHE:
        _NC_CACHE[key] = _build_nc(*key)
    return _NC_CACHE[key]


def _host_prep(l_fmap, r_fmap):
    l = np.asarray(l_fmap, dtype=np.float32)
    r = np.asarray(r_fmap, dtype=np.float32)
    l = l * np.float32(1.0 / C)  # exact: C is a power of two
    lr = np.empty(l.shape[:3] + (WLR,), dtype=np.float16)
    lr[..., :W] = l
    lr[..., W + RPAD_L:W + RPAD_L + W] = r
    lr[..., W:W + RPAD_L] = 0.0
    lr[..., W + RPAD_L + W:] = 0.0
    return lr


def _install_ntff_hook_shim(so_path="/opt/axon/libaxon_pjrt.so"):
    """Provide antenv.axon_hooks.get_axon_ntff_profile_hook via ctypes when
    the image's antenv lacks it (mirrors trn_agent_boot's slim hook)."""
    import types
    import ctypes
    import contextlib

    try:
        from antenv.axon_hooks import get_axon_ntff_profile_hook  # noqa: F401
        return
    except ImportError:
        pass

    lib = ctypes.CDLL(so_path)
    if not hasattr(lib, "axon_start_nrt_profile"):
        return
    lib.axon_start_nrt_profile.argtypes = [
        ctypes.POINTER(ctypes.c_int64), ctypes.c_size_t,
    ]
    lib.axon_start_nrt_profile.restype = ctypes.c_int64
    lib.axon_stop_nrt_profile.argtypes = [ctypes.c_char_p]
    lib.axon_stop_nrt_profile.restype = ctypes.c_int64

    @contextlib.contextmanager
    def _hook(output_dir, device_ids):
        import jax
        jax.devices()
        if device_ids:
            ids = (ctypes.c_int64 * len(device_ids))(*device_ids)
            rc = lib.axon_start_nrt_profile(ids, len(device_ids))
        else:
            rc = lib.axon_start_nrt_profile(None, 0)
        if rc != 0:
            raise RuntimeError(f"axon_start_nrt_profile rc={rc}")
        try:
            yield
        finally:
            n = lib.axon_stop_nrt_profile(str(output_dir).encode())
            print(f"ntff profile: {n} file(s) written to {output_dir}",
                  file=sys.stderr)

    import antenv
    mod = types.ModuleType("antenv.axon_hooks")
    mod.get_axon_ntff_profile_hook = lambda: _hook
    mod.set_axon_ntff_profile_hook = lambda h: None
    sys.modules["antenv.axon_hooks"] = mod
    antenv.axon_hooks = mod


def kernel(l_fmap, r_fmap, max_disp):
    global LAST_RESULTS
    assert int(max_disp) == MAXD
    lr = _host_prep(l_fmap, r_fmap)
    assert lr.shape == (B, C, H, WLR)

    nc = _get_nc()
    in_maps = []
    for k in range(NCORES):
        sl = slice(k * HS, (k + 1) * HS)
        in_maps.append({
            "lr": np.ascontiguousarray(lr[:, :, sl, :]),
        })

    import os
    trace = bool(int(os.environ.get("CV_TRACE", "0")))
    if trace:
        _install_ntff_hook_shim()
    res = run_bass_kernel_spmd(nc, in_maps, list(range(NCORES)), trace=trace)
    LAST_RESULTS = res

    out = np.empty((B, D, H, W), dtype=np.float32)
    for k in range(NCORES):
        o = np.asarray(res.results[k]["o"]).astype(np.float32)  # [B,HS,WB,OROW]
        o5 = np.stack(
            [o[..., 127 + GW * q:127 + GW * q + D] for q in range(NQ)], axis=2
        )  # [B, HS, NQ, WB, D]
        # out[b, 96-k', h, 128q+i] = o5[b, h, q, i, k']
        tmp = np.flip(o5, axis=4).transpose(0, 4, 1, 2, 3)  # [B,D,HS,NQ,WB]
        out[:, :, k * HS:(k + 1) * HS, :] = tmp.reshape(B, D, HS, W)
    return out



# revision 3
# speedup vs baseline: 1.0841x; 1.0841x over previous
"""CostVolume2D Trainium2 kernel (v2: skewed-PSUM compact store).

out[b, d, h, w] = mean_c l[b,c,h,w] * r[b,c,h, w - (d - maxd)]  (zero padded)

Strategy (8 NeuronCores, shard H — no halo since shifts only touch W):
  * Per (b, h): disparity planes are diagonals of banded gram matrices
    G[w, w'] = sum_c l[c, w] r[c, w'] with |w - w'| <= 48.
  * v2 change vs v1: instead of computing full [128 x 224] gram blocks
    and storing all 224 columns per row (2.31x write amplification),
    each 128-row block is split into 128/WG row-groups; the matmul for
    group g streams only the needed column window [WG*g, WG*g+WG+96)
    and writes it to a PSUM window shared across groups. The diagonal
    band is then pre-aligned per group in PSUM, the eviction copies the
    full 128-partition tile at full lane utilization, and the store DMA
    absorbs the residual per-row skew (+1/row within a group) with a
    3-dim flat-stride access pattern. Write amplification drops to
    (WG+96)/97 and the quadrant matmuls (K=64, M=WG) run concurrently
    on disjoint PE array tiles (tile_position auto-derived from the
    lhsT/out base partitions).
  * 4 h-rows are packed per store DMA so descriptors are 8*(WG+96)
    bytes; loads use a host-side relayout so each load descriptor is
    4 h-pairs x 1120 elems (8960 B) contiguous.
  * Host pre-divides l by C (exact, power of two) and pre-pads r along
    W so no on-device scaling / memset / edge handling.
"""

import sys

try:
    import concourse  # noqa: F401
except ImportError:
    sys.path.insert(0, "/opt/trn_rl_repo")

import numpy as np

from concourse import bass, mybir
from concourse import tile
from concourse.ap import AP
from concourse.bass_utils import run_bass_kernel_spmd

F32 = mybir.dt.float32
F16 = mybir.dt.float16

# Problem dims (hardcoded per spec)
B, C, H, W = 4, 64, 256, 512
MAXD = 48
D = 2 * MAXD + 1          # 97 disparity planes
NCORES = 8
HS = H // NCORES          # 32 h-rows per core

# Derived tiling constants
WG = 32                   # matmul row-group (M); window = WG + 96 cols
NG = 128 // WG            # row-groups per 128-row w-block
GW = WG + 2 * MAXD        # gram window width per group
NQ = W // 128             # 4 w-blocks of 128 rows
RPAD_L = MAXD             # left zero pad of r
WP = W + 2 * MAXD         # 608 padded r width
WLR = W + WP              # 1120: combined (l | r_pad) row width
RW = NQ * GW              # per-h-row store width (4 q-windows)
SROW = 4 * RW + 1         # skewed DRAM row pitch (4 h-rows + 1)
QDS = 128 * SROW          # per-(b,quad) DRAM region, elems
PSW = NQ * GW             # PSUM tile free width per h-row

# module-level result stash (test.py reads these)
LAST_RESULTS = None
_NC_CACHE = {}


def _build_nc(b_n=B, hs=HS):
    """Build the per-core Bass program. All cores run the same program."""
    nc = bass.Bass()
    npairs = hs // 2          # 16 h-pairs per core
    nquads = hs // 4          # 8 quads (4 h-rows each)
    # lr: [b, hh, c, pair, WLR] — l and r_pad concatenated on W so one
    # load DMA per (b, h-group, hh) covers 4 h-pairs contiguously.
    lr_in = nc.dram_tensor("lr", [b_n, 2, C, npairs, WLR], F16,
                           kind="ExternalInput")
    o_out = nc.dram_tensor("o", [b_n, nquads, QDS], F16,
                           kind="ExternalOutput")

    lr_hh = C * npairs * WLR
    lr_c = npairs * WLR
    lr_b = 2 * lr_hh

    with tile.TileContext(nc) as tc:
        with (
            tc.tile_pool(name="lrpool", bufs=3) as lrp,
            tc.tile_pool(name="gpool", bufs=3) as gp,
            tc.tile_pool(name="ppool", bufs=8, space="PSUM") as pp,
        ):
            eng2 = None  # set lazily (engines exist on nc)
            ld_cnt = 0
            st_cnt = 0
            for b in range(b_n):
                for hg in range(npairs // 4):   # 4 h-pairs per load group
                    t0 = hg * 4
                    lr_t = lrp.tile([128, 4 * WLR], F16, name="lr_t")
                    for hh in range(2):
                        lr_src = AP(
                            lr_in,
                            b * lr_b + hh * lr_hh + t0 * WLR,
                            [(lr_c, C), (1, 4 * WLR)],
                        )
                        eng = (nc.sync, nc.scalar)[ld_cnt % 2]
                        ld_cnt += 1
                        eng.dma_start(
                            out=lr_t[64 * hh:64 * hh + 64, :], in_=lr_src
                        )
                    for qd in range(2):         # 2 quads per load group
                        g_q = gp.tile([128, 4 * RW], F16, name="g_q")
                        for p in range(2):      # h-pair within quad
                            t = 2 * qd + p      # pair idx within group
                            ps0 = pp.tile([128, PSW], F32, name="ps0",
                                          tag="ps")
                            ps1 = pp.tile([128, PSW], F32, name="ps1",
                                          tag="ps")
                            pss = (ps0, ps1)
                            for q in range(NQ):
                                for hh in range(2):
                                    for g in range(NG):
                                        c0 = t * WLR + 128 * q + WG * g
                                        lhsT = lr_t[
                                            64 * hh:64 * hh + 64,
                                            c0:c0 + WG,
                                        ]
                                        rhs = lr_t[
                                            64 * hh:64 * hh + 64,
                                            W + c0:W + c0 + GW,
                                        ]
                                        nc.tensor.matmul(
                                            pss[hh][WG * g:WG * g + WG,
                                                    GW * q:GW * q + GW],
                                            lhsT, rhs,
                                            start=True, stop=True,
                                            tile_position=(64 * hh, WG * g),
                                        )
                            for hh in range(2):
                                hq = 2 * p + hh
                                dst = g_q[:, hq * RW:hq * RW + RW]
                                if hh == 0:
                                    nc.vector.tensor_copy(dst, pss[hh][:, :])
                                else:
                                    nc.scalar.copy(dst, pss[hh][:, :])
                        # Skew store: one DMA per quad; DRAM row pitch
                        # SROW = 4*RW+1 shifts row i by +i within its
                        # WG-row group; groups tile the region exactly.
                        d_ap = AP(
                            o_out,
                            (b * nquads + hg * 2 + qd) * QDS,
                            [(WG * SROW, NG), (SROW, WG), (1, 4 * RW)],
                        )
                        eng = (nc.scalar, nc.sync)[st_cnt % 2]
                        st_cnt += 1
                        eng.dma_start(out=d_ap, in_=g_q[:, :])
    _split_multi_waits(nc)
    return nc


def _split_multi_waits(nc):
    """The 64-byte TPB instruction encoding holds a single semaphore wait;
    walrus codegen rejects instructions whose sync_info carries more. Hoist
    all but one wait onto standalone InstEventSemaphore instructions placed
    immediately before, on the same engine (FIFO order preserves semantics).
    """
    for bb in nc.main_func.blocks:
        new_list = []
        changed = False
        for ins in bb.instructions:
            si = ins.sync_info
            if si is not None and len(si.on_wait) > 1:
                for w in list(si.on_wait)[:-1]:
                    ev = mybir.InstEventSemaphore(
                        name=nc.get_next_instruction_name(),
                        engine=ins.engine,
                        ins=[],
                        outs=[],
                        sync_info=mybir.SyncInfo(on_wait=[w], on_update=[]),
                    )
                    new_list.append(ev)
                ins.sync_info = mybir.SyncInfo(
                    on_wait=[list(si.on_wait)[-1]], on_update=list(si.on_update)
                )
                changed = True
            new_list.append(ins)
        if changed:
            bb.instructions = new_list


def _get_nc(key=(B, HS)):
    if key not in _NC_CACHE:
        _NC_CACHE[key] = _build_nc(*key)
    return _NC_CACHE[key]


def _host_prep(l_fmap, r_fmap):
    """Build lr[b, hh, c, pair, WLR] f16 with l scaled by 1/C and r padded.
    pair runs over all H//2 rows; per-core slices are taken afterwards."""
    l = np.asarray(l_fmap, dtype=np.float32) * np.float32(1.0 / C)
    r = np.asarray(r_fmap, dtype=np.float32)
    l16 = l.astype(np.float16).reshape(B, C, H // 2, 2, W)
    r16 = r.astype(np.float16).reshape(B, C, H // 2, 2, W)
    big = np.zeros((B, 2, C, H // 2, WLR), dtype=np.float16)
    big[..., :W] = l16.transpose(0, 3, 1, 2, 4)
    big[..., W + RPAD_L:W + RPAD_L + W] = r16.transpose(0, 3, 1, 2, 4)
    return big


def _install_ntff_hook_shim(so_path="/opt/axon/libaxon_pjrt.so"):
    """Provide antenv.axon_hooks.get_axon_ntff_profile_hook via ctypes when
    the image's antenv lacks it (mirrors trn_agent_boot's slim hook)."""
    import types
    import ctypes
    import contextlib

    try:
        from antenv.axon_hooks import get_axon_ntff_profile_hook  # noqa: F401
        return
    except ImportError:
        pass

    lib = ctypes.CDLL(so_path)
    if not hasattr(lib, "axon_start_nrt_profile"):
        return
    lib.axon_start_nrt_profile.argtypes = [
        ctypes.POINTER(ctypes.c_int64), ctypes.c_size_t,
    ]
    lib.axon_start_nrt_profile.restype = ctypes.c_int64
    lib.axon_stop_nrt_profile.argtypes = [ctypes.c_char_p]
    lib.axon_stop_nrt_profile.restype = ctypes.c_int64

    @contextlib.contextmanager
    def _hook(output_dir, device_ids):
        import jax
        jax.devices()
        if device_ids:
            ids = (ctypes.c_int64 * len(device_ids))(*device_ids)
            rc = lib.axon_start_nrt_profile(ids, len(device_ids))
        else:
            rc = lib.axon_start_nrt_profile(None, 0)
        if rc != 0:
            raise RuntimeError(f"axon_start_nrt_profile rc={rc}")
        try:
            yield
        finally:
            n = lib.axon_stop_nrt_profile(str(output_dir).encode())
            print(f"ntff profile: {n} file(s) written to {output_dir}",
                  file=sys.stderr)

    import antenv
    mod = types.ModuleType("antenv.axon_hooks")
    mod.get_axon_ntff_profile_hook = lambda: _hook
    mod.set_axon_ntff_profile_hook = lambda h: None
    sys.modules["antenv.axon_hooks"] = mod
    antenv.axon_hooks = mod


def kernel(l_fmap, r_fmap, max_disp):
    global LAST_RESULTS
    assert int(max_disp) == MAXD
    big = _host_prep(l_fmap, r_fmap)   # [B, 2, C, H//2, WLR]

    nc = _get_nc()
    npairs = HS // 2
    in_maps = []
    for k in range(NCORES):
        sl = slice(k * npairs, (k + 1) * npairs)
        in_maps.append({
            "lr": np.ascontiguousarray(big[:, :, :, sl, :]),
        })

    import os
    trace = bool(int(os.environ.get("CV_TRACE", "0")))
    if trace:
        _install_ntff_hook_shim()
    res = run_bass_kernel_spmd(nc, in_maps, list(range(NCORES)), trace=trace)
    LAST_RESULTS = res

    nquads = HS // 4
    out = np.empty((B, D, H, W), dtype=np.float32)
    es = 1  # element stride unit
    for k in range(NCORES):
        o = np.asarray(res.results[k]["o"]).reshape(-1)  # [B*nquads*QDS] f16
        # view axes: (b, qd, g, hq, q, i', delta), delta = k' - i' in [0, D)
        v = np.lib.stride_tricks.as_strided(
            o,
            shape=(B, nquads, NG, 4, NQ, WG, D),
            strides=tuple(np.array([
                nquads * QDS, QDS, WG * SROW, RW, GW, SROW + 1, es,
            ]) * o.itemsize),
        )
        # plane index = 96 - delta; h = 4*qd + hq; w = 128*q + WG*g + i'
        t = np.flip(v, axis=6).transpose(0, 6, 1, 3, 4, 2, 5)
        out[:, :, k * HS:(k + 1) * HS, :] = (
            t.reshape(B, D, HS, W).astype(np.float32)
        )
    return out


# revision 6
# speedup vs baseline: 1.2488x; 1.1520x over previous
"""CostVolume2D Trainium2 kernel (v2: skewed-PSUM compact store).

out[b, d, h, w] = mean_c l[b,c,h,w] * r[b,c,h, w - (d - maxd)]  (zero padded)

Strategy (8 NeuronCores, shard H — no halo since shifts only touch W):
  * Per (b, h): disparity planes are diagonals of banded gram matrices
    G[w, w'] = sum_c l[c, w] r[c, w'] with |w - w'| <= 48.
  * v2 change vs v1: instead of computing full [128 x 224] gram blocks
    and storing all 224 columns per row (2.31x write amplification),
    each 128-row block is split into 128/WG row-groups; the matmul for
    group g streams only the needed column window [WG*g, WG*g+WG+96)
    and writes it to a PSUM window shared across groups. The diagonal
    band is then pre-aligned per group in PSUM, the eviction copies the
    full 128-partition tile at full lane utilization, and the store DMA
    absorbs the residual per-row skew (+1/row within a group) with a
    3-dim flat-stride access pattern. Write amplification drops to
    (WG+96)/97 and the quadrant matmuls (K=64, M=WG) run concurrently
    on disjoint PE array tiles (tile_position auto-derived from the
    lhsT/out base partitions).
  * 4 h-rows are packed per store DMA so descriptors are 8*(WG+96)
    bytes; loads use a host-side relayout so each load descriptor is
    4 h-pairs x 1120 elems (8960 B) contiguous.
  * Host pre-divides l by C (exact, power of two) and pre-pads r along
    W so no on-device scaling / memset / edge handling.
"""

import sys

try:
    import concourse  # noqa: F401
except ImportError:
    sys.path.insert(0, "/opt/trn_rl_repo")

import numpy as np

from concourse import bass, mybir
from concourse import tile
from concourse.ap import AP
from concourse.bass_utils import run_bass_kernel_spmd

F32 = mybir.dt.float32
F16 = mybir.dt.float16

# Problem dims (hardcoded per spec)
B, C, H, W = 4, 64, 256, 512
MAXD = 48
D = 2 * MAXD + 1          # 97 disparity planes
NCORES = 8
HS = H // NCORES          # 32 h-rows per core

# Derived tiling constants
WG = 32                   # matmul row-group (M); window = WG + 96 cols
NG = 128 // WG            # row-groups per 128-row w-block
GW = WG + 2 * MAXD        # gram window width per group
NQ = W // 128             # 4 w-blocks of 128 rows
RPAD_L = MAXD             # left zero pad of r
WP = W + 2 * MAXD         # 608 padded r width
WLR = W + WP              # 1120: combined (l | r_pad) row width
RW = NQ * GW              # per-h-row store width (4 q-windows)
HPS = 8                   # h-rows packed per store DMA descriptor row
SROW = HPS * RW + 1       # skewed DRAM row pitch (8 h-rows + 1)
QDS = 128 * SROW          # per-(b,oct) DRAM region, elems
PSW = NQ * GW             # PSUM tile free width per h-row

# module-level result stash (test.py reads these)
LAST_RESULTS = None
_NC_CACHE = {}


def _build_nc(b_n=B, hs=HS):
    """Build the per-core Bass program. All cores run the same program."""
    nc = bass.Bass()
    npairs = hs // 2          # 16 h-pairs per core
    nocts = hs // HPS         # 4 octs (8 h-rows each)
    # lr: [b, hh, c, pair, WLR] — l and r_pad concatenated on W; one load
    # DMA per (b, 8-pair half) covers all 128 partitions (both hh) with
    # 8*WLR-elem (17920 B) descriptors.
    lr_in = nc.dram_tensor("lr", [b_n, 2, C, npairs, WLR], F16,
                           kind="ExternalInput")
    o_out = nc.dram_tensor("o", [b_n, nocts, QDS], F16,
                           kind="ExternalOutput")

    lr_hh = C * npairs * WLR
    lr_c = npairs * WLR
    lr_b = 2 * lr_hh

    with tile.TileContext(nc) as tc:
        with (
            tc.tile_pool(name="lrpool", bufs=3) as lrp,
            tc.tile_pool(name="gpool", bufs=3) as gp,
            tc.tile_pool(name="ppool", bufs=8, space="PSUM") as pp,
        ):
            ld_cnt = 0
            st_cnt = 0
            for b in range(b_n):
                for half in range(2):           # 8 h-pairs per load
                    t0 = half * 8
                    lr_t = lrp.tile([128, 8 * WLR], F16, name="lr_t")
                    lr_src = AP(
                        lr_in,
                        b * lr_b + t0 * WLR,
                        [(lr_hh, 2), (lr_c, C), (1, 8 * WLR)],
                    )
                    eng = (nc.sync, nc.scalar)[ld_cnt % 2]
                    ld_cnt += 1
                    eng.dma_start(out=lr_t[:, :], in_=lr_src)
                    for st in range(2):         # 2 octs per load
                        g_t = gp.tile([128, HPS * RW], F16, name="g_t")
                        for p4 in range(4):     # h-pair within oct
                            lp = st * 4 + p4    # pair idx within tile
                            ps0 = pp.tile([128, PSW], F32, name="ps0",
                                          tag="ps")
                            ps1 = pp.tile([128, PSW], F32, name="ps1",
                                          tag="ps")
                            pss = (ps0, ps1)
                            for q in range(NQ):
                                for hh in range(2):
                                    for g in range(NG):
                                        c0 = lp * WLR + 128 * q + WG * g
                                        lhsT = lr_t[
                                            64 * hh:64 * hh + 64,
                                            c0:c0 + WG,
                                        ]
                                        rhs = lr_t[
                                            64 * hh:64 * hh + 64,
                                            W + c0:W + c0 + GW,
                                        ]
                                        nc.tensor.matmul(
                                            pss[hh][WG * g:WG * g + WG,
                                                    GW * q:GW * q + GW],
                                            lhsT, rhs,
                                            start=True, stop=True,
                                            tile_position=(64 * hh, WG * g),
                                        )
                            for hh in range(2):
                                hq = 2 * p4 + hh
                                dst = g_t[:, hq * RW:hq * RW + RW]
                                if hh == 0:
                                    nc.vector.tensor_copy(dst, pss[hh][:, :])
                                else:
                                    nc.scalar.copy(dst, pss[hh][:, :])
                        # Skew store: one DMA per oct (8 h-rows); DRAM row
                        # pitch SROW = 8*RW+1 shifts row i by +i within its
                        # WG-row group; groups tile the region exactly.
                        d_ap = AP(
                            o_out,
                            (b * nocts + half * 2 + st) * QDS,
                            [(WG * SROW, NG), (SROW, WG), (1, HPS * RW)],
                        )
                        eng = (nc.scalar, nc.sync)[st_cnt % 2]
                        st_cnt += 1
                        eng.dma_start(out=d_ap, in_=g_t[:, :])
    _split_multi_waits(nc)
    return nc


def _split_multi_waits(nc):
    """The 64-byte TPB instruction encoding holds a single semaphore wait;
    walrus codegen rejects instructions whose sync_info carries more. Hoist
    all but one wait onto standalone InstEventSemaphore instructions placed
    immediately before, on the same engine (FIFO order preserves semantics).
    """
    for bb in nc.main_func.blocks:
        new_list = []
        changed = False
        for ins in bb.instructions:
            si = ins.sync_info
            if si is not None and len(si.on_wait) > 1:
                for w in list(si.on_wait)[:-1]:
                    ev = mybir.InstEventSemaphore(
                        name=nc.get_next_instruction_name(),
                        engine=ins.engine,
                        ins=[],
                        outs=[],
                        sync_info=mybir.SyncInfo(on_wait=[w], on_update=[]),
                    )
                    new_list.append(ev)
                ins.sync_info = mybir.SyncInfo(
                    on_wait=[list(si.on_wait)[-1]], on_update=list(si.on_update)
                )
                changed = True
            new_list.append(ins)
        if changed:
            bb.instructions = new_list


def _get_nc(key=(B, HS)):
    if key not in _NC_CACHE:
        _NC_CACHE[key] = _build_nc(*key)
    return _NC_CACHE[key]


def _host_prep(l_fmap, r_fmap):
    """Build lr[b, hh, c, pair, WLR] f16 with l scaled by 1/C and r padded.
    pair runs over all H//2 rows; per-core slices are taken afterwards."""
    l = np.asarray(l_fmap, dtype=np.float32) * np.float32(1.0 / C)
    r = np.asarray(r_fmap, dtype=np.float32)
    l16 = l.astype(np.float16).reshape(B, C, H // 2, 2, W)
    r16 = r.astype(np.float16).reshape(B, C, H // 2, 2, W)
    big = np.zeros((B, 2, C, H // 2, WLR), dtype=np.float16)
    big[..., :W] = l16.transpose(0, 3, 1, 2, 4)
    big[..., W + RPAD_L:W + RPAD_L + W] = r16.transpose(0, 3, 1, 2, 4)
    return big


def _install_ntff_hook_shim(so_path="/opt/axon/libaxon_pjrt.so"):
    """Provide antenv.axon_hooks.get_axon_ntff_profile_hook via ctypes when
    the image's antenv lacks it (mirrors trn_agent_boot's slim hook)."""
    import types
    import ctypes
    import contextlib

    try:
        from antenv.axon_hooks import get_axon_ntff_profile_hook  # noqa: F401
        return
    except ImportError:
        pass

    lib = ctypes.CDLL(so_path)
    if not hasattr(lib, "axon_start_nrt_profile"):
        return
    lib.axon_start_nrt_profile.argtypes = [
        ctypes.POINTER(ctypes.c_int64), ctypes.c_size_t,
    ]
    lib.axon_start_nrt_profile.restype = ctypes.c_int64
    lib.axon_stop_nrt_profile.argtypes = [ctypes.c_char_p]
    lib.axon_stop_nrt_profile.restype = ctypes.c_int64

    @contextlib.contextmanager
    def _hook(output_dir, device_ids):
        import jax
        jax.devices()
        if device_ids:
            ids = (ctypes.c_int64 * len(device_ids))(*device_ids)
            rc = lib.axon_start_nrt_profile(ids, len(device_ids))
        else:
            rc = lib.axon_start_nrt_profile(None, 0)
        if rc != 0:
            raise RuntimeError(f"axon_start_nrt_profile rc={rc}")
        try:
            yield
        finally:
            n = lib.axon_stop_nrt_profile(str(output_dir).encode())
            print(f"ntff profile: {n} file(s) written to {output_dir}",
                  file=sys.stderr)

    import antenv
    mod = types.ModuleType("antenv.axon_hooks")
    mod.get_axon_ntff_profile_hook = lambda: _hook
    mod.set_axon_ntff_profile_hook = lambda h: None
    sys.modules["antenv.axon_hooks"] = mod
    antenv.axon_hooks = mod


def kernel(l_fmap, r_fmap, max_disp):
    global LAST_RESULTS
    assert int(max_disp) == MAXD
    big = _host_prep(l_fmap, r_fmap)   # [B, 2, C, H//2, WLR]

    nc = _get_nc()
    npairs = HS // 2
    in_maps = []
    for k in range(NCORES):
        sl = slice(k * npairs, (k + 1) * npairs)
        in_maps.append({
            "lr": np.ascontiguousarray(big[:, :, :, sl, :]),
        })

    import os
    trace = bool(int(os.environ.get("CV_TRACE", "0")))
    if trace:
        _install_ntff_hook_shim()
    res = run_bass_kernel_spmd(nc, in_maps, list(range(NCORES)), trace=trace)
    LAST_RESULTS = res

    nocts = HS // HPS
    out = np.empty((B, D, H, W), dtype=np.float32)
    for k in range(NCORES):
        o = np.asarray(res.results[k]["o"]).reshape(-1)  # [B*nocts*QDS] f16
        # view axes: (b, oct, g, hq, q, i', delta), delta = k' - i' in [0, D)
        v = np.lib.stride_tricks.as_strided(
            o,
            shape=(B, nocts, NG, HPS, NQ, WG, D),
            strides=tuple(np.array([
                nocts * QDS, QDS, WG * SROW, RW, GW, SROW + 1, 1,
            ]) * o.itemsize),
        )
        # plane index = 96 - delta; h = 8*oct + hq; w = 128*q + WG*g + i'
        t = np.flip(v, axis=6).transpose(0, 6, 1, 3, 4, 2, 5)
        out[:, :, k * HS:(k + 1) * HS, :] = (
            t.reshape(B, D, HS, W).astype(np.float32)
        )
    return out


# revision 9
# speedup vs baseline: 1.3029x; 1.0433x over previous
"""CostVolume2D Trainium2 kernel (v2: skewed-PSUM compact store).

out[b, d, h, w] = mean_c l[b,c,h,w] * r[b,c,h, w - (d - maxd)]  (zero padded)

Strategy (8 NeuronCores, shard H — no halo since shifts only touch W):
  * Per (b, h): disparity planes are diagonals of banded gram matrices
    G[w, w'] = sum_c l[c, w] r[c, w'] with |w - w'| <= 48.
  * v2 change vs v1: instead of computing full [128 x 224] gram blocks
    and storing all 224 columns per row (2.31x write amplification),
    each 128-row block is split into 128/WG row-groups; the matmul for
    group g streams only the needed column window [WG*g, WG*g+WG+96)
    and writes it to a PSUM window shared across groups. The diagonal
    band is then pre-aligned per group in PSUM, the eviction copies the
    full 128-partition tile at full lane utilization, and the store DMA
    absorbs the residual per-row skew (+1/row within a group) with a
    3-dim flat-stride access pattern. Write amplification drops to
    (WG+96)/97 and the quadrant matmuls (K=64, M=WG) run concurrently
    on disjoint PE array tiles (tile_position auto-derived from the
    lhsT/out base partitions).
  * 4 h-rows are packed per store DMA so descriptors are 8*(WG+96)
    bytes; loads use a host-side relayout so each load descriptor is
    4 h-pairs x 1120 elems (8960 B) contiguous.
  * Host pre-divides l by C (exact, power of two) and pre-pads r along
    W so no on-device scaling / memset / edge handling.
"""

import sys

try:
    import concourse  # noqa: F401
except ImportError:
    sys.path.insert(0, "/opt/trn_rl_repo")

import numpy as np

from concourse import bass, mybir
from concourse import tile
from concourse.ap import AP
from concourse.bass_utils import run_bass_kernel_spmd

F32 = mybir.dt.float32
F16 = mybir.dt.float16

# Problem dims (hardcoded per spec)
B, C, H, W = 4, 64, 256, 512
MAXD = 48
D = 2 * MAXD + 1          # 97 disparity planes
NCORES = 8
HS = H // NCORES          # 32 h-rows per core

# Derived tiling constants
WG = 32                   # matmul row-group (M); window = WG + 96 cols
NG = 128 // WG            # row-groups per 128-row w-block
GW = WG + 2 * MAXD        # gram window width per group
NQ = W // 128             # 4 w-blocks of 128 rows
RPAD_L = MAXD             # left zero pad of r
WP = W + 2 * MAXD         # 608 padded r width
WLR = W + WP              # 1120: combined (l | r_pad) row width
RW = NQ * GW              # per-h-row store width (4 q-windows)
HPS = 8                   # h-rows packed per store DMA descriptor row
SROW = HPS * RW + 1       # skewed DRAM row pitch (8 h-rows + 1)
QDS = 128 * SROW          # per-(b,oct) DRAM region, elems
PSW = NQ * GW             # PSUM tile free width per h-row

# module-level result stash (test.py reads these)
LAST_RESULTS = None
_NC_CACHE = {}


def _build_nc(b_n=B, hs=HS):
    """Build the per-core Bass program. All cores run the same program."""
    nc = bass.Bass()
    npairs = hs // 2          # 16 h-pairs per core
    nocts = hs // HPS         # 4 octs (8 h-rows each)
    # lr: [b, hh, c, pair, WLR] — l and r_pad concatenated on W; one load
    # DMA per (b, 8-pair half) covers all 128 partitions (both hh) with
    # 8*WLR-elem (17920 B) descriptors.
    lr_in = nc.dram_tensor("lr", [b_n, 2, C, npairs, WLR], F16,
                           kind="ExternalInput")
    o_out = nc.dram_tensor("o", [b_n, nocts, QDS], F16,
                           kind="ExternalOutput")

    lr_hh = C * npairs * WLR
    lr_c = npairs * WLR
    lr_b = 2 * lr_hh

    with tile.TileContext(nc) as tc:
        with (
            tc.tile_pool(name="lrpool", bufs=4) as lrp,
            tc.tile_pool(name="gpool", bufs=4) as gp,
            tc.tile_pool(name="ppool", bufs=8, space="PSUM") as pp,
        ):
            for b in range(b_n):
                for half in range(2):           # 8 h-pairs per load
                    t0 = half * 8
                    lr_t = lrp.tile([128, 8 * WLR], F16, name="lr_t")
                    lr_src = AP(
                        lr_in,
                        b * lr_b + t0 * WLR,
                        [(lr_hh, 2), (lr_c, C), (1, 8 * WLR)],
                    )
                    # All loads issue from sync: its FIFO holds only loads,
                    # so prefetch is never head-of-line blocked by a store
                    # waiting on evictions (stores live on scalar's FIFO).
                    nc.sync.dma_start(out=lr_t[:, :], in_=lr_src)
                    for st in range(2):         # 2 octs per load
                        g_t = gp.tile([128, HPS * RW], F16, name="g_t")
                        for p4 in range(4):     # h-pair within oct
                            lp = st * 4 + p4    # pair idx within tile
                            ps0 = pp.tile([128, PSW], F32, name="ps0",
                                          tag="ps")
                            ps1 = pp.tile([128, PSW], F32, name="ps1",
                                          tag="ps")
                            pss = (ps0, ps1)
                            for q in range(NQ):
                                for hh in range(2):
                                    for g in range(NG):
                                        c0 = lp * WLR + 128 * q + WG * g
                                        lhsT = lr_t[
                                            64 * hh:64 * hh + 64,
                                            c0:c0 + WG,
                                        ]
                                        rhs = lr_t[
                                            64 * hh:64 * hh + 64,
                                            W + c0:W + c0 + GW,
                                        ]
                                        nc.tensor.matmul(
                                            pss[hh][WG * g:WG * g + WG,
                                                    GW * q:GW * q + GW],
                                            lhsT, rhs,
                                            start=True, stop=True,
                                            tile_position=(64 * hh, WG * g),
                                        )
                            for hh in range(2):
                                hq = 2 * p4 + hh
                                dst = g_t[:, hq * RW:hq * RW + RW]
                                if hh == 0:
                                    nc.vector.tensor_copy(dst, pss[hh][:, :])
                                else:
                                    nc.scalar.copy(dst, pss[hh][:, :])
                        # Skew store: one DMA per oct (8 h-rows); DRAM row
                        # pitch SROW = 8*RW+1 shifts row i by +i within its
                        # WG-row group; groups tile the region exactly.
                        d_ap = AP(
                            o_out,
                            (b * nocts + half * 2 + st) * QDS,
                            [(WG * SROW, NG), (SROW, WG), (1, HPS * RW)],
                        )
                        nc.scalar.dma_start(out=d_ap, in_=g_t[:, :])
    _split_multi_waits(nc)
    return nc


def _split_multi_waits(nc):
    """The 64-byte TPB instruction encoding holds a single semaphore wait;
    walrus codegen rejects instructions whose sync_info carries more. Hoist
    all but one wait onto standalone InstEventSemaphore instructions placed
    immediately before, on the same engine (FIFO order preserves semantics).
    """
    for bb in nc.main_func.blocks:
        new_list = []
        changed = False
        for ins in bb.instructions:
            si = ins.sync_info
            if si is not None and len(si.on_wait) > 1:
                for w in list(si.on_wait)[:-1]:
                    ev = mybir.InstEventSemaphore(
                        name=nc.get_next_instruction_name(),
                        engine=ins.engine,
                        ins=[],
                        outs=[],
                        sync_info=mybir.SyncInfo(on_wait=[w], on_update=[]),
                    )
                    new_list.append(ev)
                ins.sync_info = mybir.SyncInfo(
                    on_wait=[list(si.on_wait)[-1]], on_update=list(si.on_update)
                )
                changed = True
            new_list.append(ins)
        if changed:
            bb.instructions = new_list


def _get_nc(key=(B, HS)):
    if key not in _NC_CACHE:
        _NC_CACHE[key] = _build_nc(*key)
    return _NC_CACHE[key]


def _host_prep(l_fmap, r_fmap):
    """Build lr[b, hh, c, pair, WLR] f16 with l scaled by 1/C and r padded.
    pair runs over all H//2 rows; per-core slices are taken afterwards."""
    l = np.asarray(l_fmap, dtype=np.float32) * np.float32(1.0 / C)
    r = np.asarray(r_fmap, dtype=np.float32)
    l16 = l.astype(np.float16).reshape(B, C, H // 2, 2, W)
    r16 = r.astype(np.float16).reshape(B, C, H // 2, 2, W)
    big = np.zeros((B, 2, C, H // 2, WLR), dtype=np.float16)
    big[..., :W] = l16.transpose(0, 3, 1, 2, 4)
    big[..., W + RPAD_L:W + RPAD_L + W] = r16.transpose(0, 3, 1, 2, 4)
    return big


def _install_ntff_hook_shim(so_path="/opt/axon/libaxon_pjrt.so"):
    """Provide antenv.axon_hooks.get_axon_ntff_profile_hook via ctypes when
    the image's antenv lacks it (mirrors trn_agent_boot's slim hook)."""
    import types
    import ctypes
    import contextlib

    try:
        from antenv.axon_hooks import get_axon_ntff_profile_hook  # noqa: F401
        return
    except ImportError:
        pass

    lib = ctypes.CDLL(so_path)
    if not hasattr(lib, "axon_start_nrt_profile"):
        return
    lib.axon_start_nrt_profile.argtypes = [
        ctypes.POINTER(ctypes.c_int64), ctypes.c_size_t,
    ]
    lib.axon_start_nrt_profile.restype = ctypes.c_int64
    lib.axon_stop_nrt_profile.argtypes = [ctypes.c_char_p]
    lib.axon_stop_nrt_profile.restype = ctypes.c_int64

    @contextlib.contextmanager
    def _hook(output_dir, device_ids):
        import jax
        jax.devices()
        if device_ids:
            ids = (ctypes.c_int64 * len(device_ids))(*device_ids)
            rc = lib.axon_start_nrt_profile(ids, len(device_ids))
        else:
            rc = lib.axon_start_nrt_profile(None, 0)
        if rc != 0:
            raise RuntimeError(f"axon_start_nrt_profile rc={rc}")
        try:
            yield
        finally:
            n = lib.axon_stop_nrt_profile(str(output_dir).encode())
            print(f"ntff profile: {n} file(s) written to {output_dir}",
                  file=sys.stderr)

    import antenv
    mod = types.ModuleType("antenv.axon_hooks")
    mod.get_axon_ntff_profile_hook = lambda: _hook
    mod.set_axon_ntff_profile_hook = lambda h: None
    sys.modules["antenv.axon_hooks"] = mod
    antenv.axon_hooks = mod


def kernel(l_fmap, r_fmap, max_disp):
    global LAST_RESULTS
    assert int(max_disp) == MAXD
    big = _host_prep(l_fmap, r_fmap)   # [B, 2, C, H//2, WLR]

    nc = _get_nc()
    npairs = HS // 2
    in_maps = []
    for k in range(NCORES):
        sl = slice(k * npairs, (k + 1) * npairs)
        in_maps.append({
            "lr": np.ascontiguousarray(big[:, :, :, sl, :]),
        })

    import os
    trace = bool(int(os.environ.get("CV_TRACE", "0")))
    if trace:
        _install_ntff_hook_shim()
    res = run_bass_kernel_spmd(nc, in_maps, list(range(NCORES)), trace=trace)
    LAST_RESULTS = res

    nocts = HS // HPS
    out = np.empty((B, D, H, W), dtype=np.float32)
    for k in range(NCORES):
        o = np.asarray(res.results[k]["o"]).reshape(-1)  # [B*nocts*QDS] f16
        # view axes: (b, oct, g, hq, q, i', delta), delta = k' - i' in [0, D)
        v = np.lib.stride_tricks.as_strided(
            o,
            shape=(B, nocts, NG, HPS, NQ, WG, D),
            strides=tuple(np.array([
                nocts * QDS, QDS, WG * SROW, RW, GW, SROW + 1, 1,
            ]) * o.itemsize),
        )
        # plane index = 96 - delta; h = 8*oct + hq; w = 128*q + WG*g + i'
        t = np.flip(v, axis=6).transpose(0, 6, 1, 3, 4, 2, 5)
        out[:, :, k * HS:(k + 1) * HS, :] = (
            t.reshape(B, D, HS, W).astype(np.float32)
        )
    return out


# revision 14
# speedup vs baseline: 1.3617x; 1.0452x over previous
"""CostVolume2D Trainium2 kernel (v2: skewed-PSUM compact store).

out[b, d, h, w] = mean_c l[b,c,h,w] * r[b,c,h, w - (d - maxd)]  (zero padded)

Strategy (8 NeuronCores, shard H — no halo since shifts only touch W):
  * Per (b, h): disparity planes are diagonals of banded gram matrices
    G[w, w'] = sum_c l[c, w] r[c, w'] with |w - w'| <= 48.
  * v2 change vs v1: instead of computing full [128 x 224] gram blocks
    and storing all 224 columns per row (2.31x write amplification),
    each 128-row block is split into 128/WG row-groups; the matmul for
    group g streams only the needed column window [WG*g, WG*g+WG+96)
    and writes it to a PSUM window shared across groups. The diagonal
    band is then pre-aligned per group in PSUM, the eviction copies the
    full 128-partition tile at full lane utilization, and the store DMA
    absorbs the residual per-row skew (+1/row within a group) with a
    3-dim flat-stride access pattern. Write amplification drops to
    (WG+96)/97 and the quadrant matmuls (K=64, M=WG) run concurrently
    on disjoint PE array tiles (tile_position auto-derived from the
    lhsT/out base partitions).
  * 4 h-rows are packed per store DMA so descriptors are 8*(WG+96)
    bytes; loads use a host-side relayout so each load descriptor is
    4 h-pairs x 1120 elems (8960 B) contiguous.
  * Host pre-divides l by C (exact, power of two) and pre-pads r along
    W so no on-device scaling / memset / edge handling.
"""

import sys

try:
    import concourse  # noqa: F401
except ImportError:
    sys.path.insert(0, "/opt/trn_rl_repo")

import numpy as np

from concourse import bass, mybir
from concourse import tile
from concourse.ap import AP
from concourse.bass_utils import run_bass_kernel_spmd

F32 = mybir.dt.float32
F16 = mybir.dt.float16

# Problem dims (hardcoded per spec)
B, C, H, W = 4, 64, 256, 512
MAXD = 48
D = 2 * MAXD + 1          # 97 disparity planes
NCORES = 8
HS = H // NCORES          # 32 h-rows per core

# Derived tiling constants
WG = 64                   # matmul row-group (M); window = WG + 96 cols
NG = 128 // WG            # row-groups per 128-row w-block
GW = WG + 2 * MAXD        # gram window width per group
NQ = W // 128             # 4 w-blocks of 128 rows
RPAD_L = MAXD             # left zero pad of r
WP = W + 2 * MAXD         # 608 padded r width
WLR = W + WP              # 1120: combined (l | r_pad) row width
RW = NQ * GW              # per-h-row store width (4 q-windows)
HPS = 8                   # h-rows packed per store DMA descriptor row
SROW = HPS * RW + 1       # skewed DRAM row pitch (8 h-rows + 1)
QDS = 128 * SROW          # per-(b,oct) DRAM region, elems
# PSUM q-window offsets (elems) — each window must stay inside one 2 KiB
# bank (512 f32); for WG=64 (GW=160) q2/q3 move up to the second bank.
if WG == 32:
    PSW = NQ * GW         # 512: one bank
    QOFF = [GW * q for q in range(NQ)]
else:
    PSW = 1024            # two banks
    QOFF = [0, 160, 512, 672]

# module-level result stash (test.py reads these)
LAST_RESULTS = None
_NC_CACHE = {}


def _build_nc(b_n=B, hs=HS):
    """Build the per-core Bass program. All cores run the same program."""
    nc = bass.Bass()
    npairs = hs // 2          # 16 h-pairs per core
    nocts = hs // HPS         # 4 octs (8 h-rows each)
    # lr: [b, hh, c, pair, WLR] — l and r_pad concatenated on W; one load
    # DMA per (b, 8-pair half) covers all 128 partitions (both hh) with
    # 8*WLR-elem (17920 B) descriptors.
    lr_in = nc.dram_tensor("lr", [b_n, 2, C, npairs, WLR], F16,
                           kind="ExternalInput")
    o_out = nc.dram_tensor("o", [b_n, nocts, QDS], F16,
                           kind="ExternalOutput")

    lr_hh = C * npairs * WLR
    lr_c = npairs * WLR
    lr_b = 2 * lr_hh

    with tile.TileContext(nc) as tc:
        with (
            tc.tile_pool(name="lrpool", bufs=4) as lrp,
            tc.tile_pool(name="gpool", bufs=4) as gp,
            tc.tile_pool(name="ppool", bufs=(8 if WG == 32 else 4),
                         space="PSUM") as pp,
        ):
            for b in range(b_n):
                for half in range(2):           # 8 h-pairs per load
                    t0 = half * 8
                    lr_t = lrp.tile([128, 8 * WLR], F16, name="lr_t")
                    lr_src = AP(
                        lr_in,
                        b * lr_b + t0 * WLR,
                        [(lr_hh, 2), (lr_c, C), (1, 8 * WLR)],
                    )
                    # All loads issue from sync: its FIFO holds only loads,
                    # so prefetch is never head-of-line blocked by a store
                    # waiting on evictions (stores live on scalar's FIFO).
                    nc.sync.dma_start(out=lr_t[:, :], in_=lr_src)
                    for st in range(2):         # 2 octs per load
                        g_t = gp.tile([128, HPS * RW], F16, name="g_t")
                        for p4 in range(4):     # h-pair within oct
                            lp = st * 4 + p4    # pair idx within tile
                            ps0 = pp.tile([128, PSW], F32, name="ps0",
                                          tag="ps")
                            ps1 = pp.tile([128, PSW], F32, name="ps1",
                                          tag="ps")
                            pss = (ps0, ps1)
                            for q in range(NQ):
                                for hh in range(2):
                                    for g in range(NG):
                                        c0 = lp * WLR + 128 * q + WG * g
                                        lhsT = lr_t[
                                            64 * hh:64 * hh + 64,
                                            c0:c0 + WG,
                                        ]
                                        rhs = lr_t[
                                            64 * hh:64 * hh + 64,
                                            W + c0:W + c0 + GW,
                                        ]
                                        nc.tensor.matmul(
                                            pss[hh][WG * g:WG * g + WG,
                                                    QOFF[q]:QOFF[q] + GW],
                                            lhsT, rhs,
                                            start=True, stop=True,
                                            tile_position=(64 * hh, WG * g),
                                        )
                            for hh in range(2):
                                hq = 2 * p4 + hh
                                dst = g_t[:, hq * RW:hq * RW + RW]
                                eng_copy = (
                                    nc.vector.tensor_copy if hh == 0
                                    else nc.scalar.copy
                                )
                                if WG == 32:
                                    eng_copy(dst, pss[hh][:, :])
                                else:
                                    # q0/q1 in bank 0, q2/q3 in bank 1
                                    eng_copy(
                                        g_t[:, hq * RW:hq * RW + 2 * GW],
                                        pss[hh][:, 0:2 * GW],
                                    )
                                    eng_copy(
                                        g_t[:, hq * RW + 2 * GW:
                                            hq * RW + RW],
                                        pss[hh][:, 512:512 + 2 * GW],
                                    )
                        # Skew store: one DMA per oct (8 h-rows); DRAM row
                        # pitch SROW = 8*RW+1 shifts row i by +i within its
                        # WG-row group; groups tile the region exactly.
                        d_ap = AP(
                            o_out,
                            (b * nocts + half * 2 + st) * QDS,
                            [(WG * SROW, NG), (SROW, WG), (1, HPS * RW)],
                        )
                        nc.scalar.dma_start(out=d_ap, in_=g_t[:, :])
    _split_multi_waits(nc)
    return nc


def _split_multi_waits(nc):
    """The 64-byte TPB instruction encoding holds a single semaphore wait;
    walrus codegen rejects instructions whose sync_info carries more. Hoist
    all but one wait onto standalone InstEventSemaphore instructions placed
    immediately before, on the same engine (FIFO order preserves semantics).
    """
    for bb in nc.main_func.blocks:
        new_list = []
        changed = False
        for ins in bb.instructions:
            si = ins.sync_info
            if si is not None and len(si.on_wait) > 1:
                for w in list(si.on_wait)[:-1]:
                    ev = mybir.InstEventSemaphore(
                        name=nc.get_next_instruction_name(),
                        engine=ins.engine,
                        ins=[],
                        outs=[],
                        sync_info=mybir.SyncInfo(on_wait=[w], on_update=[]),
                    )
                    new_list.append(ev)
                ins.sync_info = mybir.SyncInfo(
                    on_wait=[list(si.on_wait)[-1]], on_update=list(si.on_update)
                )
                changed = True
            new_list.append(ins)
        if changed:
            bb.instructions = new_list


def _get_nc(key=(B, HS)):
    if key not in _NC_CACHE:
        _NC_CACHE[key] = _build_nc(*key)
    return _NC_CACHE[key]


def _host_prep(l_fmap, r_fmap):
    """Build lr[b, hh, c, pair, WLR] f16 with l scaled by 1/C and r padded.
    pair runs over all H//2 rows; per-core slices are taken afterwards."""
    l = np.asarray(l_fmap, dtype=np.float32) * np.float32(1.0 / C)
    r = np.asarray(r_fmap, dtype=np.float32)
    l16 = l.astype(np.float16).reshape(B, C, H // 2, 2, W)
    r16 = r.astype(np.float16).reshape(B, C, H // 2, 2, W)
    big = np.zeros((B, 2, C, H // 2, WLR), dtype=np.float16)
    big[..., :W] = l16.transpose(0, 3, 1, 2, 4)
    big[..., W + RPAD_L:W + RPAD_L + W] = r16.transpose(0, 3, 1, 2, 4)
    return big


def _install_ntff_hook_shim(so_path="/opt/axon/libaxon_pjrt.so"):
    """Provide antenv.axon_hooks.get_axon_ntff_profile_hook via ctypes when
    the image's antenv lacks it (mirrors trn_agent_boot's slim hook)."""
    import types
    import ctypes
    import contextlib

    try:
        from antenv.axon_hooks import get_axon_ntff_profile_hook  # noqa: F401
        return
    except ImportError:
        pass

    lib = ctypes.CDLL(so_path)
    if not hasattr(lib, "axon_start_nrt_profile"):
        return
    lib.axon_start_nrt_profile.argtypes = [
        ctypes.POINTER(ctypes.c_int64), ctypes.c_size_t,
    ]
    lib.axon_start_nrt_profile.restype = ctypes.c_int64
    lib.axon_stop_nrt_profile.argtypes = [ctypes.c_char_p]
    lib.axon_stop_nrt_profile.restype = ctypes.c_int64

    @contextlib.contextmanager
    def _hook(output_dir, device_ids):
        import jax
        jax.devices()
        if device_ids:
            ids = (ctypes.c_int64 * len(device_ids))(*device_ids)
            rc = lib.axon_start_nrt_profile(ids, len(device_ids))
        else:
            rc = lib.axon_start_nrt_profile(None, 0)
        if rc != 0:
            raise RuntimeError(f"axon_start_nrt_profile rc={rc}")
        try:
            yield
        finally:
            n = lib.axon_stop_nrt_profile(str(output_dir).encode())
            print(f"ntff profile: {n} file(s) written to {output_dir}",
                  file=sys.stderr)

    import antenv
    mod = types.ModuleType("antenv.axon_hooks")
    mod.get_axon_ntff_profile_hook = lambda: _hook
    mod.set_axon_ntff_profile_hook = lambda h: None
    sys.modules["antenv.axon_hooks"] = mod
    antenv.axon_hooks = mod


def kernel(l_fmap, r_fmap, max_disp):
    global LAST_RESULTS
    assert int(max_disp) == MAXD
    big = _host_prep(l_fmap, r_fmap)   # [B, 2, C, H//2, WLR]

    nc = _get_nc()
    npairs = HS // 2
    in_maps = []
    for k in range(NCORES):
        sl = slice(k * npairs, (k + 1) * npairs)
        in_maps.append({
            "lr": np.ascontiguousarray(big[:, :, :, sl, :]),
        })

    import os
    trace = bool(int(os.environ.get("CV_TRACE", "0")))
    if trace:
        _install_ntff_hook_shim()
    res = run_bass_kernel_spmd(nc, in_maps, list(range(NCORES)), trace=trace)
    LAST_RESULTS = res

    nocts = HS // HPS
    out = np.empty((B, D, H, W), dtype=np.float32)
    for k in range(NCORES):
        o = np.asarray(res.results[k]["o"]).reshape(-1)  # [B*nocts*QDS] f16
        # view axes: (b, oct, g, hq, q, i', delta), delta = k' - i' in [0, D)
        v = np.lib.stride_tricks.as_strided(
            o,
            shape=(B, nocts, NG, HPS, NQ, WG, D),
            strides=tuple(np.array([
                nocts * QDS, QDS, WG * SROW, RW, GW, SROW + 1, 1,
            ]) * o.itemsize),
        )
        # plane index = 96 - delta; h = 8*oct + hq; w = 128*q + WG*g + i'
        t = np.flip(v, axis=6).transpose(0, 6, 1, 3, 4, 2, 5)
        out[:, :, k * HS:(k + 1) * HS, :] = (
            t.reshape(B, D, HS, W).astype(np.float32)
        )
    return out
